# revision 8
# baseline (speedup 1.0000x reference)
"""ConvergedInhibition forward on 8 Trainium2 NeuronCores.

The reference computes, independently for every (n, h, w) pixel, a
frequency-domain deconvolution along the channel axis C=128:

    out = ifft(fft(x, axis=C) / Fk).real

Division by Fk in frequency space is circular convolution with
g = ifft(1/Fk) (real, since delta-k is real), i.e. a fixed 128x128
circulant matrix M applied to every channel vector:

    out[n, :, h, w] = M @ x[n, :, h, w],   M[c, c'] = g[(c - c') mod C]

So the heavy work is a tiny stationary matmul swept over a 134 MB
activation tensor -> memory-bound tensor-engine kernel. The length-128
filter preprocessing (FFT of a 128-vector) is negligible and done on
host in float64.

Sharding: data-parallel over batch N=64 -> 8 batches per core, no
cross-core communication. Each core: DMA (128, 4096) batch slices to
SBUF, matmul against the stationary inverse-circulant lhsT in 512-col
PSUM-bank chunks, copy PSUM->SBUF (alternating vector/scalar engines),
DMA back.
"""

import numpy as np

import concourse.bass as bass
import concourse.mybir as mybir
from concourse import bacc
from concourse.bass_utils import run_bass_kernel_spmd
from concourse.tile import TileContext

N_CORES = 8
PSUM_CHUNK = 512  # fp32 elements per PSUM bank


def _inverse_circulant_lhsT(filt: np.ndarray, C: int) -> np.ndarray:
    """Build the stationary matmul operand lhsT (K x M layout).

    out[m] = sum_k M[m, k] x[k] with M[m, k] = g[(m - k) mod C], and the
    tensor engine computes lhsT.T @ rhs, so lhsT[k, m] = g[(m - k) mod C].
    """
    scope = filt.shape[-1]
    pad_left = (C - scope) // 2
    k = np.zeros(C, dtype=np.float64)
    k[pad_left : pad_left + scope] = filt.reshape(-1).astype(np.float64)
    k = np.roll(k, C // 2 + 1)
    delta = np.zeros(C, dtype=np.float64)
    delta[0] = 1.0
    g = np.fft.ifft(1.0 / np.fft.fft(delta - k)).real
    j = np.arange(C)
    return g[(j[None, :] - j[:, None]) % C].astype(np.float32)


def build_nc(b_per_core: int, C: int, P: int, use_f32r: bool = False) -> bacc.Bacc:
    # float32r streams fp32 bits through the PE in a single reduced-mantissa
    # pass (1 cycle/row at N>=512) instead of fp32's two half-speed passes.
    mm_dt = mybir.dt.float32r if use_f32r else mybir.dt.float32
    nc = bacc.Bacc("TRN2", target_bir_lowering=False, debug=False)
    x = nc.dram_tensor("x", [b_per_core, C, P], mm_dt, kind="ExternalInput")
    w = nc.dram_tensor("w", [C, C], mm_dt, kind="ExternalInput")
    y = nc.dram_tensor("y", [b_per_core, C, P], mybir.dt.float32, kind="ExternalOutput")

    n_chunks = P // PSUM_CHUNK

    with TileContext(nc) as tc:
        with (
            tc.tile_pool(name="wp", bufs=1) as wp,
            tc.tile_pool(name="xp", bufs=3) as xp,
            tc.tile_pool(name="yp", bufs=3) as yp,
            tc.tile_pool(name="pp", bufs=8, space="PSUM") as pp,
        ):
            wt = wp.tile([C, C], mm_dt)
            nc.sync.dma_start(wt[:], w[:, :])
            for b in range(b_per_core):
                xt = xp.tile([C, P], mm_dt)
                nc.sync.dma_start(xt[:], x[b, :, :])
                yt = yp.tile([C, P], mybir.dt.float32)
                for j in range(n_chunks):
                    pt = pp.tile([C, PSUM_CHUNK], mybir.dt.float32)
                    cols = bass.ts(j, PSUM_CHUNK)
                    nc.tensor.matmul(
                        pt[:], wt[:], xt[:, cols], start=True, stop=True
                    )
                    # PSUM has no DMA route; split the drain across both
                    # copy-capable engines so neither becomes the bottleneck.
                    if j % 2 == 0:
                        nc.vector.tensor_copy(yt[:, cols], pt[:])
                    else:
                        nc.scalar.copy(yt[:, cols], pt[:])
                nc.sync.dma_start(y[b, :, :], yt[:])
    nc.compile()
    return nc


def _run(activations, inhibition_filter, use_f32r=False, **spmd_kwargs):
    act = np.ascontiguousarray(np.asarray(activations, dtype=np.float32))
    filt = np.asarray(inhibition_filter, dtype=np.float32)
    B, C, H, W = act.shape
    P = H * W
    assert B % N_CORES == 0
    b_per_core = B // N_CORES

    lhsT = _inverse_circulant_lhsT(filt, C)
    nc = build_nc(b_per_core, C, P, use_f32r=use_f32r)

    xs = act.reshape(N_CORES, b_per_core, C, P)
    in_maps = [{"x": xs[i], "w": lhsT} for i in range(N_CORES)]
    res = run_bass_kernel_spmd(nc, in_maps, core_ids=list(range(N_CORES)), **spmd_kwargs)
    out = np.stack([res.results[i]["y"] for i in range(N_CORES)], axis=0)
    return out.reshape(B, C, H, W), res


def kernel(activations: np.ndarray, inhibition_filter: np.ndarray) -> np.ndarray:
    out, _ = _run(activations, inhibition_filter)
    return out


# revision 9
# speedup vs baseline: 1.1593x; 1.1593x over previous
"""ConvergedInhibition forward on 8 Trainium2 NeuronCores.

The reference computes, independently for every (n, h, w) pixel, a
frequency-domain deconvolution along the channel axis C=128:

    out = ifft(fft(x, axis=C) / Fk).real

Division by Fk in frequency space is circular convolution with
g = ifft(1/Fk) (real, since delta-k is real), i.e. a fixed 128x128
circulant matrix M applied to every channel vector:

    out[n, :, h, w] = M @ x[n, :, h, w],   M[c, c'] = g[(c - c') mod C]

So the heavy work is a tiny stationary matmul swept over a 134 MB
activation tensor -> memory-bound tensor-engine kernel. The length-128
filter preprocessing (FFT of a 128-vector) is negligible and done on
host in float64.

Sharding: data-parallel over batch N=64 -> 8 batches per core, no
cross-core communication. Each core: DMA (128, 4096) batch slices to
SBUF, matmul against the stationary inverse-circulant lhsT in 512-col
PSUM-bank chunks, copy PSUM->SBUF (alternating vector/scalar engines),
DMA back.
"""

import numpy as np

import concourse.bass as bass
import concourse.mybir as mybir
from concourse import bacc
from concourse.bass_utils import run_bass_kernel_spmd
from concourse.tile import TileContext

N_CORES = 8
PSUM_CHUNK = 512  # fp32 elements per PSUM bank


def _inverse_circulant_lhsT(filt: np.ndarray, C: int) -> np.ndarray:
    """Build the stationary matmul operand lhsT (K x M layout).

    out[m] = sum_k M[m, k] x[k] with M[m, k] = g[(m - k) mod C], and the
    tensor engine computes lhsT.T @ rhs, so lhsT[k, m] = g[(m - k) mod C].
    """
    scope = filt.shape[-1]
    pad_left = (C - scope) // 2
    k = np.zeros(C, dtype=np.float64)
    k[pad_left : pad_left + scope] = filt.reshape(-1).astype(np.float64)
    k = np.roll(k, C // 2 + 1)
    delta = np.zeros(C, dtype=np.float64)
    delta[0] = 1.0
    g = np.fft.ifft(1.0 / np.fft.fft(delta - k)).real
    j = np.arange(C)
    return g[(j[None, :] - j[:, None]) % C].astype(np.float32)


def build_nc(
    b_per_core: int, C: int, P: int, use_f32r: bool = False, half: int = 2048
) -> bacc.Bacc:
    # float32r streams fp32 bits through the PE in a single reduced-mantissa
    # pass (1 cycle/row at N>=512) instead of fp32's two half-speed passes.
    # Measured: no e2e gain (DMA-paced kernel) and ~1e-4 rel err, so fp32
    # stays the default.
    mm_dt = mybir.dt.float32r if use_f32r else mybir.dt.float32
    nc = bacc.Bacc("TRN2", target_bir_lowering=False, debug=False)
    x = nc.dram_tensor("x", [b_per_core, C, P], mm_dt, kind="ExternalInput")
    w = nc.dram_tensor("w", [C, C], mm_dt, kind="ExternalInput")
    y = nc.dram_tensor("y", [b_per_core, C, P], mybir.dt.float32, kind="ExternalOutput")

    n_halves = P // half          # 1 MB sub-tiles: fine-grained pipeline
    n_chunks = half // PSUM_CHUNK

    with TileContext(nc) as tc:
        with (
            tc.tile_pool(name="wp", bufs=1) as wp,
            tc.tile_pool(name="xp", bufs=6) as xp,
            tc.tile_pool(name="yp", bufs=6) as yp,
            tc.tile_pool(name="pp", bufs=8, space="PSUM") as pp,
        ):
            wt = wp.tile([C, C], mm_dt)
            nc.sync.dma_start(wt[:], w[:, :])
            for b in range(b_per_core):
                for h in range(n_halves):
                    xt = xp.tile([C, half], mm_dt, tag="x")
                    nc.sync.dma_start(xt[:], x[b, :, bass.ts(h, half)])
                    yt = yp.tile([C, half], mybir.dt.float32, tag="y")
                    for j in range(n_chunks):
                        pt = pp.tile([C, PSUM_CHUNK], mybir.dt.float32)
                        cols = bass.ts(j, PSUM_CHUNK)
                        nc.tensor.matmul(
                            pt[:], wt[:], xt[:, cols], start=True, stop=True
                        )
                        # PSUM has no DMA route: drain via both copy engines —
                        # early chunks on DVE, late on ACT, so the ACT-queue
                        # out-DMA below follows its inputs mostly in program
                        # order instead of a cross-engine wait.
                        if j < n_chunks // 2:
                            nc.vector.tensor_copy(yt[:, cols], pt[:])
                        else:
                            nc.scalar.copy(yt[:, cols], pt[:])
                    # Out-DMAs ride the scalar engine's own HWDGE queue so a
                    # pending output never head-of-line blocks input loads on
                    # the sync queue.
                    nc.scalar.dma_start(y[b, :, bass.ts(h, half)], yt[:])
    nc.compile()
    return nc


def _run(activations, inhibition_filter, use_f32r=False, **spmd_kwargs):
    act = np.ascontiguousarray(np.asarray(activations, dtype=np.float32))
    filt = np.asarray(inhibition_filter, dtype=np.float32)
    B, C, H, W = act.shape
    P = H * W
    assert B % N_CORES == 0
    b_per_core = B // N_CORES

    lhsT = _inverse_circulant_lhsT(filt, C)
    nc = build_nc(b_per_core, C, P, use_f32r=use_f32r)

    xs = act.reshape(N_CORES, b_per_core, C, P)
    in_maps = [{"x": xs[i], "w": lhsT} for i in range(N_CORES)]
    res = run_bass_kernel_spmd(nc, in_maps, core_ids=list(range(N_CORES)), **spmd_kwargs)
    out = np.stack([res.results[i]["y"] for i in range(N_CORES)], axis=0)
    return out.reshape(B, C, H, W), res


def kernel(activations: np.ndarray, inhibition_filter: np.ndarray) -> np.ndarray:
    out, _ = _run(activations, inhibition_filter)
    return out


# revision 12
# speedup vs baseline: 1.3014x; 1.1226x over previous
"""ConvergedInhibition forward on 8 Trainium2 NeuronCores.

The reference computes, independently for every (n, h, w) pixel, a
frequency-domain deconvolution along the channel axis C=128:

    out = ifft(fft(x, axis=C) / Fk).real

Division by Fk in frequency space is circular convolution with
g = ifft(1/Fk) (real, since delta-k is real), i.e. a fixed 128x128
circulant matrix M applied to every channel vector:

    out[n, :, h, w] = M @ x[n, :, h, w],   M[c, c'] = g[(c - c') mod C]

So the heavy work is a tiny stationary matmul swept over a 134 MB
activation tensor -> memory-bound tensor-engine kernel. The length-128
filter preprocessing (FFT of a 128-vector) is negligible and done on
host in float64.

Sharding: data-parallel over batch N=64 -> 8 batches per core, no
cross-core communication. Each core streams (128, 2048) 1 MB half-tiles:
HWDGE DMA in on the sync queue, matmul against the stationary
inverse-circulant lhsT in 512-col PSUM-bank chunks, drain PSUM->SBUF on
both copy engines, DMA out on the scalar engine's HWDGE queue (so
pending outputs never head-of-line block input loads). Measured on HW:
~105 us/core vs a ~94 us HBM roofline (33.6 MB/core at 358 GB/s).
"""

import numpy as np

import concourse.bass as bass
import concourse.mybir as mybir
from concourse import bacc
from concourse.bass_utils import run_bass_kernel_spmd
from concourse.tile import TileContext

N_CORES = 8
PSUM_CHUNK = 512  # fp32 elements per PSUM bank


def _inverse_circulant_lhsT(filt: np.ndarray, C: int) -> np.ndarray:
    """Build the stationary matmul operand lhsT (K x M layout).

    out[m] = sum_k M[m, k] x[k] with M[m, k] = g[(m - k) mod C], and the
    tensor engine computes lhsT.T @ rhs, so lhsT[k, m] = g[(m - k) mod C].
    """
    scope = filt.shape[-1]
    pad_left = (C - scope) // 2
    k = np.zeros(C, dtype=np.float64)
    k[pad_left : pad_left + scope] = filt.reshape(-1).astype(np.float64)
    k = np.roll(k, C // 2 + 1)
    delta = np.zeros(C, dtype=np.float64)
    delta[0] = 1.0
    g = np.fft.ifft(1.0 / np.fft.fft(delta - k)).real
    j = np.arange(C)
    return g[(j[None, :] - j[:, None]) % C].astype(np.float32)


def build_nc(
    b_per_core: int, C: int, P: int, use_f32r: bool = False, half: int = 2048
) -> bacc.Bacc:
    # float32r streams fp32 bits through the PE in a single reduced-mantissa
    # pass (1 cycle/row at N>=512) instead of fp32's two half-speed passes.
    # Measured: no e2e gain (DMA-paced kernel) and ~1e-4 rel err, so fp32
    # stays the default.
    mm_dt = mybir.dt.float32r if use_f32r else mybir.dt.float32
    nc = bacc.Bacc("TRN2", target_bir_lowering=False, debug=False)
    x = nc.dram_tensor("x", [b_per_core, C, P], mm_dt, kind="ExternalInput")
    w = nc.dram_tensor("w", [C, C], mm_dt, kind="ExternalInput")
    y = nc.dram_tensor("y", [b_per_core, C, P], mybir.dt.float32, kind="ExternalOutput")

    n_halves = P // half          # 1 MB sub-tiles: fine-grained pipeline
    n_chunks = half // PSUM_CHUNK

    with TileContext(nc) as tc:
        with (
            tc.tile_pool(name="wp", bufs=1) as wp,
            tc.tile_pool(name="xp", bufs=8) as xp,
            tc.tile_pool(name="yp", bufs=8) as yp,
            tc.tile_pool(name="pp", bufs=8, space="PSUM") as pp,
        ):
            wt = wp.tile([C, C], mm_dt)
            nc.sync.dma_start(wt[:], w[:, :])
            for b in range(b_per_core):
                for h in range(n_halves):
                    xt = xp.tile([C, half], mm_dt, tag="x")
                    nc.sync.dma_start(xt[:], x[b, :, bass.ts(h, half)])
                    yt = yp.tile([C, half], mybir.dt.float32, tag="y")
                    for j in range(n_chunks):
                        pt = pp.tile([C, PSUM_CHUNK], mybir.dt.float32)
                        cols = bass.ts(j, PSUM_CHUNK)
                        nc.tensor.matmul(
                            pt[:], wt[:], xt[:, cols], start=True, stop=True
                        )
                        # PSUM has no DMA route: drain via both copy engines —
                        # early chunks on DVE, late on ACT, so the ACT-queue
                        # out-DMA below follows its inputs mostly in program
                        # order instead of a cross-engine wait.
                        if j < n_chunks // 2:
                            nc.vector.tensor_copy(yt[:, cols], pt[:])
                        else:
                            nc.scalar.copy(yt[:, cols], pt[:])
                    # Out-DMAs ride the scalar engine's own HWDGE queue so a
                    # pending output never head-of-line blocks input loads on
                    # the sync queue.
                    nc.scalar.dma_start(y[b, :, bass.ts(h, half)], yt[:])
    nc.compile()
    return nc


_NC_CACHE: dict = {}


def _run(activations, inhibition_filter, use_f32r=False, **spmd_kwargs):
    act = np.ascontiguousarray(np.asarray(activations, dtype=np.float32))
    filt = np.asarray(inhibition_filter, dtype=np.float32)
    B, C, H, W = act.shape
    P = H * W
    assert B % N_CORES == 0
    b_per_core = B // N_CORES

    lhsT = _inverse_circulant_lhsT(filt, C)
    key = (b_per_core, C, P, use_f32r)
    nc = _NC_CACHE.get(key)
    if nc is None:
        nc = _NC_CACHE[key] = build_nc(b_per_core, C, P, use_f32r=use_f32r)

    xs = act.reshape(N_CORES, b_per_core, C, P)
    in_maps = [{"x": xs[i], "w": lhsT} for i in range(N_CORES)]
    res = run_bass_kernel_spmd(nc, in_maps, core_ids=list(range(N_CORES)), **spmd_kwargs)
    out = np.stack([res.results[i]["y"] for i in range(N_CORES)], axis=0)
    return out.reshape(B, C, H, W), res


def kernel(activations: np.ndarray, inhibition_filter: np.ndarray) -> np.ndarray:
    out, _ = _run(activations, inhibition_filter)
    return out
